# revision 15
# baseline (speedup 1.0000x reference)
"""Lookahead depthwise convolution on 8 Trainium2 NeuronCores.

out[t, b, f] = sum_{c=0..K-1} x[t+c, b, f] * weight[f, c], zero-padded at the
right edge. x: (2048, 32, 1280) fp32, weight: (1280, 81) fp32.

Strategy: shard the (fully independent) feature dim across 8 cores, 160
features each. Per feature the time conv is a banded Toeplitz matmul: with
128-wide time tiles, out_j = A_f @ x_j + B_f @ x_{j+1} where (as lhsT, i.e.
contraction index m first)
  A_f[m, t] = w[f, m - t]        (0 <= m - t < K)
  B_f[m, t] = w[f, m + 128 - t]  (0 <= m + 128 - t < K)

v2 vs v1 (379 us):
 - x is cast to fp16 on the host and shipped pre-transposed per core as
   (half, s, f, b) with f split in two halves of 80 -> input DMA halves and
   the on-chip fp32->fp16 cast disappears.
 - output is produced in fp16 in the same (half, s, f, b) layout (host
   transposes back and upcasts) -> output DMA halves and the PSUM eviction
   copy becomes stride-1 in its innermost dim.
 - matmuls cover a 4-block window in the free dim (N=128/96/32 instead of
   16x N=32) so each LDWEIGHTS is amortized over ~4x more streaming cycles.
 - PSUM eviction alternates between the vector and scalar engines.
"""

import numpy as np

import concourse.bass as bass
import concourse.bacc as bacc
import concourse.mybir as mybir
from concourse import tile
from concourse.bass_utils import run_bass_kernel_spmd

S, B, F, K = 2048, 32, 1280, 81
N_CORES = 8
FC = F // N_CORES          # features per core (160)
FH = FC // 2               # features per half-pass (80)
W = 4                      # time blocks (of 128) per matmul window
NW = S // (128 * W)        # windows (4)
CH = FH * B                # free elems per row chunk (2560)
G = 4                      # features per PSUM bank group
NG = FH // G               # psum groups per window (20)

_compiled = None


def _build_program():
    nc = bacc.Bacc("TRN2", target_bir_lowering=False, debug=False)
    f32, f16 = mybir.dt.float32, mybir.dt.float16

    x_in = nc.declare_dram_parameter("x", [2, S, CH], f16, isOutput=False)
    bandsA_in = nc.declare_dram_parameter("bandsA", [128, FC * 128], f16,
                                          isOutput=False)
    # B[m, t] = w[m + 128 - t] is nonzero only on partitions m < 81
    bandsB_in = nc.declare_dram_parameter("bandsB", [81, FC * 128], f16,
                                          isOutput=False)
    out_ext = nc.declare_dram_parameter("out", [2, S, CH], f16, isOutput=True)

    # (half, s, c) -> (half, window, partition, block j, c) with s =
    # (w*W + j)*128 + p
    x_r = x_in.rearrange("h (w j p) c -> h w p j c", j=W, p=128)
    out_r = out_ext.rearrange("h (w j p) c -> h w p j c", j=W, p=128)

    with tile.TileContext(nc) as tc:
        with (
            tc.tile_pool(name="zero", bufs=1) as zpool,
            tc.tile_pool(name="bandsA", bufs=4) as bApool,
            tc.tile_pool(name="bandsB", bufs=4) as bBpool,
            tc.tile_pool(name="x", bufs=3) as xpool,
            tc.tile_pool(name="stage", bufs=2) as spool,
            tc.tile_pool(name="psum", bufs=8, space="PSUM") as ppool,
        ):
            # zero rhs used to close the zero-padded final block's psum
            # columns (a second start=True would clear the whole bank)
            zero_rhs = zpool.tile([128, B], f16)
            nc.vector.memset(zero_rhs[:], 0.0)

            # bands in 4 chunks of 40 features each so the first matmuls
            # only wait on chunk 0, not the whole 8.6 MB
            CBF = FC // 4  # 40 features per band chunk
            bandA, bandB = [], []
            for ch in range(4):
                ta = bApool.tile([128, CBF * 128], f16)
                tb = bBpool.tile([81, CBF * 128], f16)
                nc.gpsimd.dma_start(
                    out=ta[:], in_=bandsA_in[:, ch * CBF * 128:
                                             (ch + 1) * CBF * 128])
                nc.gpsimd.dma_start(
                    out=tb[:], in_=bandsB_in[:, ch * CBF * 128:
                                             (ch + 1) * CBF * 128])
                bandA.append(ta)
                bandB.append(tb)

            def load_window(h, w):
                xt = xpool.tile([128, W * CH], f16)
                nc.sync.dma_start(
                    out=xt.rearrange("p (j c) -> p j c", j=W),
                    in_=x_r[h, w])
                return xt

            x_cur = load_window(0, 0)
            for h in range(2):
                for w in range(NW):
                    last = w == NW - 1
                    if not last:
                        x_nxt = load_window(h, w + 1)
                    elif h == 0:
                        x_nxt = load_window(1, 0)
                    else:
                        x_nxt = None
                    # views: free dims (j, f, b)
                    xv = x_cur.rearrange("p (j f b) -> p j f b", j=W, b=B)
                    nv = (x_nxt.rearrange("p (j f b) -> p j f b", j=W, b=B)
                          if x_nxt is not None else None)
                    stage = spool.tile([128, W * CH], f16)
                    for g in range(NG):
                        psum = ppool.tile([128, G * W * B], f32)
                        for f4 in range(G):
                            fh = g * G + f4
                            fg = h * FH + fh          # feature on this core
                            base = (fg % CBF) * 128
                            lA = bandA[fg // CBF][:, base:base + 128]
                            lB = bandB[fg // CBF][:, base:base + 128]
                            pc = psum[:, f4 * 128:(f4 + 1) * 128]
                            nc.tensor.matmul(
                                out=pc[:, 0:128], lhsT=lA,
                                rhs=xv[:, :, fh, :],
                                start=True, stop=False)
                            nc.tensor.matmul(
                                out=pc[:, 0:96], lhsT=lB,
                                rhs=xv[0:81, 1:4, fh, :],
                                start=False, stop=True)
                            # the final block's lookahead is zero-padded
                            nc.tensor.matmul(
                                out=pc[:, 96:128], lhsT=lB,
                                rhs=(nv[0:81, 0, fh, :] if not last
                                     else zero_rhs[0:81, :]),
                                start=False, stop=True)
                        # psum free layout (f4, j, b) -> stage (j, f, b)
                        pv = psum.rearrange("p (f j b) -> p j f b", f=G, j=W)
                        sv = stage.rearrange("p (j f b) -> p j f b", j=W, b=B)
                        eng = nc.vector.tensor_copy if g % 2 == 0 \
                            else nc.scalar.copy
                        eng(out=sv[:, :, g * G:(g + 1) * G, :], in_=pv)
                    nc.scalar.dma_start(
                        out=out_r[h, w],
                        in_=stage.rearrange("p (j c) -> p j c", j=W))
                    x_cur = x_nxt
    nc.finalize()
    return nc


def _build_bands(weight):
    # lhsT layout: bands[m, f, 0, t] = w[f, m-t], bands[m, f, 1, t] =
    # w[f, m+128-t]; contraction index m is the partition dim.
    m = np.arange(128)[:, None]
    t = np.arange(128)[None, :]
    dA = m - t
    dB = m + 128 - t
    mA = (dA >= 0) & (dA < K)
    mB = (dB >= 0) & (dB < K)
    iA = np.clip(dA, 0, K - 1)
    iB = np.clip(dB, 0, K - 1)
    w16 = weight.astype(np.float16).astype(np.float32)
    A = w16[:, iA] * mA          # [F, 128m, 128t]
    Bm = w16[:, iB] * mB
    bands = np.empty((128, F, 2, 128), np.float16)
    bands[:, :, 0, :] = A.transpose(1, 0, 2)
    bands[:, :, 1, :] = Bm.transpose(1, 0, 2)
    return bands


def _prep_inputs(x, weight):
    """Per-core input maps: x as fp16 (half, s, f, b); bands fp16."""
    x16 = np.ascontiguousarray(x, dtype=np.float16)
    bands = _build_bands(np.asarray(weight, dtype=np.float32))
    in_maps = []
    for c in range(N_CORES):
        fl = slice(c * FC, (c + 1) * FC)
        xc = x16[:, :, fl].reshape(S, B, 2, FH).transpose(2, 0, 3, 1)
        in_maps.append({
            "x": np.ascontiguousarray(xc).reshape(2, S, CH),
            "bandsA": np.ascontiguousarray(
                bands[:, fl, 0, :]).reshape(128, FC * 128),
            "bandsB": np.ascontiguousarray(
                bands[0:81, fl, 1, :]).reshape(81, FC * 128),
        })
    return in_maps


def _post_outputs(res):
    outs = []
    for c in range(N_CORES):
        o = np.asarray(res.results[c]["out"]).reshape(2, S, FH, B)
        outs.append(o.transpose(1, 3, 0, 2).reshape(S, B, FC))
    return np.concatenate(outs, axis=2).astype(np.float32)


def kernel(x, weight):
    global _compiled
    if _compiled is None:
        _compiled = _build_program()
    in_maps = _prep_inputs(x, weight)
    res = run_bass_kernel_spmd(_compiled, in_maps, list(range(N_CORES)))
    return _post_outputs(res)
